# revision 20
# baseline (speedup 1.0000x reference)
"""AnomalyTransformer Trainium2 kernel.

3-layer transformer encoder (d=64 -> d_model=512, N=1024 tokens, B=16),
data-parallel over batch: 8 NeuronCores x 2 batches each, weights
replicated, no collectives.  The Gaussian-prior branch of the reference
is a dead computation (never touches the output) and is skipped.

Layout strategy per core (per batch, N=1024 tokens):
  - Input is pre-transposed AND bf16-converted on host: xt [64, 2048].
  - All matmuls run in bf16 (fp32 PSUM accumulation).
  - QKV projections produce Q^T, K^T (dm-chunk partition, token free)
    and V row-major, all bf16.
  - Attention scores are computed directly TRANSPOSED: A^T[col, row] so
    that exp(A^T) tiles are immediately usable as matmul lhsT for
    Z = softmax(A) @ V without any transposes.
  - Softmax uses no max-subtraction (logits empirically bounded ~15);
    normalization is folded into the residual via one fused DVE
    scalar_tensor_tensor: zpre = (Z * 1/s) + h, which also emits the
    true row-sum (accum_out) for the LN that follows.  The FFN drain is
    the same shape: gpre = max(f, 0) + z in one fused op.
  - PSUM tiles are [128, 1024] spanning two banks; matmul chains fill
    the two 512-wide halves (each within one bank).  The fused drain op
    reads PSUM at the head of each drain chain, so banks recycle fast.
  - QK projection chains run hf-outer (token-half outer loop) so the
    first chains of layer l+1 depend only on the first half of the h^T
    transposes; Q/K/V PSUM drains alternate between DVE and ACT.
  - LayerNorm alternates two engine assignments by row parity to keep
    DVE and ACT equally loaded: the A-form computes stats with
    bn_stats/bn_aggr on the bf16 zpre (DVE), the B-form squares on ACT
    with a free fp32 accumulator and only runs tiny [128,1] chains on
    the DVE.  Both end with sqrt on ACT -> reciprocal + normalize on
    DVE (one cross-engine hop each).
  - DMA is spread over the three queues: weights + inputs + xbar
    transposes (z^T/g^T) on the sync HWDGE queue (weight prefetch must
    NOT sit on the scalar queue: its serialized DMA_DIRECT2Ds
    head-of-line block the ACT FIFO for ~10us), output stores on the
    otherwise-idle gpsimd SWDGE queue, merged to one [256,512] store
    per row-pair.  z^T transposes are enqueued per row-pair right after
    the producing LN so no queue ever faces an 8-transpose burst.
  - Software pipelining across batches: each layer's FFN for batch 0 is
    interleaved into batch 1's Z phase at row-pair granularity, and the
    FFN for batch 1 is interleaved into the NEXT layer's batch-0 QKV
    phase.  Drain chains therefore always overlap a long stretch of
    independent matmuls, and only the last layer's FFN(b1) remains as
    the kernel tail.
  - A short burst of dummy matmuls on a zeroed tile runs while the
    input DMA is in flight: the PE's HAM clock-gate needs ~3.4us of
    sustained activity to lift the default 4/8 throttle, and the burst
    buys that warm-up during otherwise-idle head time.
  - When the affine params are not identity the general bn_stats path
    with explicit affine ops is used instead (correct, less tuned).
"""

import numpy as np

import concourse.bass as bass
import concourse.mybir as mybir
import concourse.tile as tile
from concourse import bacc
from concourse.bass_utils import run_bass_kernel_spmd

F32 = mybir.dt.float32
BF16 = mybir.dt.bfloat16
TRACE = False

D0 = 64      # input feature dim
DM = 512     # d_model
NT = 1024    # tokens per batch
NB = 2       # batches per core
NCORES = 8
DC = DM // 128   # 4 dm chunks
RT = NT // 128   # 8 token tiles per batch
RP = RT // 2     # 4 row pairs per batch
HF = NT // 512   # 2 moving-operand halves
ISQ = 1.0 / float(np.sqrt(DM))
EPS = 1e-5
NWARM = 6        # dummy matmuls to lift the HAM clock throttle


def build_graph(nc, affine_identity=False, bf_zero=False):
    T = NB * NT

    d = {}
    d["xt"] = nc.declare_dram_parameter("xt", [D0, T], BF16, isOutput=False)
    for nm in ("wq0", "wk0", "wv0"):
        d[nm] = nc.declare_dram_parameter(nm, [D0, DM], BF16, isOutput=False)
    for nm, L in (("wqs", 2), ("wks", 2), ("wvs", 2), ("wf", 3)):
        d[nm] = nc.declare_dram_parameter(nm, [L, DM, DM], BF16, isOutput=False)
    for nm in ("g1", "b1", "g2", "b2", "bf"):
        d[nm] = nc.declare_dram_parameter(nm, [3, DM], F32, isOutput=False)
    # bf16 output: halves the store traffic in the kernel tail; the host
    # upcasts to f32 (adds ~0.2% element error, well inside the budget)
    d["out"] = nc.declare_dram_parameter("out", [T, DM], BF16, isOutput=True)

    with tile.TileContext(nc) as tc:
        _build_tc(tc, nc, d, affine_identity, bf_zero)
    nc.compile()
    return nc


def _build_tc(tc, nc, d, affine_identity=False, bf_zero=False):
    from contextlib import ExitStack
    ctx = ExitStack()
    with ctx:
        const = ctx.enter_context(tc.tile_pool(name="const", bufs=1))
        wpool = ctx.enter_context(tc.tile_pool(name="wpool", bufs=9))
        lnpool = ctx.enter_context(tc.tile_pool(name="lnpool", bufs=8))
        qkpool = ctx.enter_context(tc.tile_pool(name="qkpool", bufs=8))
        vpool = ctx.enter_context(tc.tile_pool(name="vpool", bufs=5))
        epool = ctx.enter_context(tc.tile_pool(name="epool", bufs=8))
        zpool = ctx.enter_context(tc.tile_pool(name="zpool", bufs=8))
        gpool = ctx.enter_context(tc.tile_pool(name="gpool", bufs=10))
        tchunk = ctx.enter_context(tc.tile_pool(name="tchunk", bufs=4))
        xpool = ctx.enter_context(tc.tile_pool(name="xpool", bufs=2))
        smallp = ctx.enter_context(tc.tile_pool(name="smallp", bufs=12))
        tmpp = ctx.enter_context(tc.tile_pool(name="tmpp", bufs=7))
        sqpool = ctx.enter_context(tc.tile_pool(name="sqpool", bufs=3))
        fpool = ctx.enter_context(tc.tile_pool(name="fpool", bufs=3))
        gout = ctx.enter_context(tc.tile_pool(name="gout", bufs=4))
        ps_big = ctx.enter_context(tc.tile_pool(name="ps_big", bufs=3,
                                                space="PSUM"))
        # two bufs: the two row-sum chains of a pair must live in separate
        # banks (a chain's first matmul clears the whole bank's has_written)
        ps_s = ctx.enter_context(tc.tile_pool(name="ps_s", bufs=2,
                                              space="PSUM"))

        # input + ALL weights on the sync HWDGE queue: the first QK chains
        # need only xt half 0 + wq0; the big weight loads land by ~25us
        # (first needed ~100us).  Keeping them OFF the scalar queue matters:
        # serialized DMA_DIRECT2Ds there head-of-line block the ACT FIFO.
        xts = [xpool.tile([D0, NT], BF16, tag="xt", name=f"xt{b}")
               for b in range(NB)]
        nc.sync.dma_start(out=xts[0][:, 0:512], in_=d["xt"][:, 0:512])
        w0 = {}
        for name in ("wq0", "wk0"):
            t = const.tile([D0, DM], BF16, tag=name)
            nc.sync.dma_start(out=t, in_=d[name][:])
            w0[name] = t
        nc.sync.dma_start(out=xts[0][:, 512:1024], in_=d["xt"][:, 512:1024])
        t = const.tile([D0, DM], BF16, tag="wv0")
        nc.sync.dma_start(out=t, in_=d["wv0"][:])
        w0["wv0"] = t
        for hf in range(HF):
            nc.sync.dma_start(
                out=xts[1][:, hf * 512:(hf + 1) * 512],
                in_=d["xt"][:, NT + hf * 512:NT + (hf + 1) * 512])

        ones = const.tile([128, 1], BF16)
        nc.vector.memset(ones, 1.0)
        eps_t = const.tile([128, 1], F32)
        nc.vector.memset(eps_t, EPS)

        # zeroed [128,512] tile: HAM warm-up operand + the "+0" second
        # input of the L0 fused drain (no residual in layer 0)
        warm = const.tile([128, 512], BF16, tag="warm")
        nc.vector.memset(warm, 0.0)
        wps = ps_big.tile([128, 1024], F32, tag="pb")
        for i in range(NWARM):
            nc.tensor.matmul(wps[:, 0:512], warm[:, 0:128], warm,
                             start=True, stop=True)

        def load_w(key, idx):
            t = wpool.tile([128, DC, DM], BF16, tag="W")
            nc.sync.dma_start(
                out=t, in_=d[key][idx].rearrange("(c p) o -> p c o", p=128))
            return t

        lws = []
        for l in range(3):
            lwd = {}
            if l > 0:
                for nm, key in (("wq", "wqs"), ("wk", "wks"), ("wv", "wvs")):
                    lwd[nm] = load_w(key, l - 1)
            lwd["wf"] = load_w("wf", l)
            lws.append(lwd)

        def load_ln(name, l):
            t = lnpool.tile([128, DM], F32, tag="ln")
            nc.scalar.dma_start(
                out=t, in_=d[name][l].unsqueeze(0).to_broadcast((128, DM)))
            return t

        IDM = 1.0 / DM
        MULT = mybir.AluOpType.mult
        ADD = mybir.AluOpType.add
        SUB = mybir.AluOpType.subtract
        MAX = mybir.AluOpType.max

        def layernorm_r(zpre, out_ap, gb, bb):
            """General-path LN (non-identity affine): bn_stats on DVE."""
            stats = smallp.tile([128, 6], F32, tag="stats")
            mv = smallp.tile([128, 2], F32, tag="mv")
            nc.vector.bn_stats(out=stats, in_=zpre)
            nc.vector.bn_aggr(out=mv, in_=stats)
            stdv = smallp.tile([128, 1], F32, tag="stdv")
            nc.scalar.activation(out=stdv, in_=mv[:, 1:2],
                                 func=mybir.ActivationFunctionType.Sqrt,
                                 bias=eps_t, scale=1.0)
            rstd = smallp.tile([128, 1], F32, tag="rstd")
            nc.vector.reciprocal(out=rstd, in_=stdv)
            nc.vector.tensor_scalar(
                out=out_ap, in0=zpre, scalar1=mv[:, 0:1], scalar2=rstd,
                op0=SUB, op1=MULT)
            if gb is not None:
                nc.vector.tensor_mul(out=out_ap, in0=out_ap, in1=gb)
                nc.vector.tensor_add(out=out_ap, in0=out_ap, in1=bb)

        POW = mybir.AluOpType.pow

        def ln_a(zpre, out_ap):
            """LN with stats on DVE and rstd = exp(-0.5*ln(var+eps)) on
            ACT.  Keeping Sqrt off the ACT engine matters a lot: Sqrt and
            Exp live in different ACT spline-table sets, and every
            alternation costs a ~1.5us ACT_TABLE_LOAD that stalls the
            whole ACT FIFO -- Log/Exp share the softmax Exp's set."""
            stats = smallp.tile([128, 6], F32, tag="stats")
            mv = smallp.tile([128, 2], F32, tag="mv")
            nc.vector.bn_stats(out=stats, in_=zpre)
            nc.vector.bn_aggr(out=mv, in_=stats)
            lv = smallp.tile([128, 1], F32, tag="lv")
            nc.scalar.activation(out=lv, in_=mv[:, 1:2],
                                 func=mybir.ActivationFunctionType.Ln,
                                 bias=eps_t, scale=1.0)
            rstd = smallp.tile([128, 1], F32, tag="rstd")
            nc.scalar.activation(out=rstd, in_=lv,
                                 func=mybir.ActivationFunctionType.Exp,
                                 scale=-0.5)
            nc.vector.tensor_scalar(
                out=out_ap, in0=zpre, scalar1=mv[:, 0:1], scalar2=rstd,
                op0=SUB, op1=MULT)

        hT = [None] * NB    # [128, DC, NT] bf16 per batch (transposed h)
        h = [None] * NB     # RP pair tiles [128, 1024] bf16 per batch
        zs = [None] * NB
        zTs = [None] * NB

        def hsl(b, r):
            return h[b][r // 2][:, (r % 2) * DM:(r % 2 + 1) * DM]

        def phase1(l, b, lw, g1b, b1b, work=None):
            """QKV + scores/exp + Z + LN1 (+ per-pair z^T transposes) for
            one batch.  `work` is an ordered list of pending FFN-pair
            thunks (or None entries to skip a slot), consumed one per
            V-pair and one per Z-pair: drain bursts always overlap a
            stretch of independent matmuls, and cross-phase inputs (z^T /
            h^T transposes) are produced long before they are needed."""
            widx = [0]

            def slot():
                if work is not None and widx[0] < len(work):
                    thunk = work[widx[0]]
                    widx[0] += 1
                    if thunk is not None:
                        thunk()
            qT = [qkpool.tile([128, NT], BF16, tag="qk", name=f"qT{o}")
                  for o in range(DC)]
            kT = [qkpool.tile([128, NT], BF16, tag="qk", name=f"kT{o}")
                  for o in range(DC)]
            v2 = [vpool.tile([128, 2 * DM], BF16, tag="v2", name=f"v{p}")
                  for p in range(RP)]

            def vsl(c):
                return v2[c // 2][:, (c % 2) * DM:(c % 2 + 1) * DM]

            # QK chains: one [128,1024] PSUM tile per dm-chunk (both token
            # halves); full-tile drains alternate DVE/ACT.  (The h^T
            # transposes these chains depend on complete with lots of
            # slack now that the producing FFN is interleaved a phase
            # earlier, so no hf-ordering tricks are needed.)
            for dst, wname in ((qT, "wq"), (kT, "wk")):
                for o in range(DC):
                    ps = ps_big.tile([128, 1024], F32, tag="pb")
                    for hf in range(HF):
                        psl = ps[:, hf * 512:(hf + 1) * 512]
                        if l == 0:
                            w0n = "wq0" if wname == "wq" else "wk0"
                            nc.tensor.matmul(
                                psl,
                                w0[w0n][:, o * 128:(o + 1) * 128],
                                xts[b][:, hf * 512:(hf + 1) * 512],
                                start=True, stop=True)
                        else:
                            for i in range(DC):
                                nc.tensor.matmul(
                                    psl,
                                    lw[wname][:, i, o * 128:(o + 1) * 128],
                                    hT[b][:, i, hf * 512:(hf + 1) * 512],
                                    start=(i == 0), stop=(i == DC - 1))
                    nc.scalar.copy(out=dst[o], in_=ps)

            for p in range(RP):
                ps = ps_big.tile([128, 1024], F32, tag="pb")
                if l == 0:
                    for j in range(2):
                        r = 2 * p + j
                        nc.tensor.matmul(
                            ps[:, j * 512:(j + 1) * 512],
                            xts[b][:, r * 128:(r + 1) * 128], w0["wv0"],
                            start=True, stop=True)
                else:
                    for i in range(DC):
                        for j in range(2):
                            r = 2 * p + j
                            nc.tensor.matmul(
                                ps[:, j * 512:(j + 1) * 512],
                                hT[b][:, i, r * 128:(r + 1) * 128],
                                lw["wv"][:, i, :],
                                start=(i == 0), stop=(i == DC - 1))
                nc.scalar.copy(out=v2[p], in_=ps)
                slot()

            eT = []
            for c in range(RT):
                et = epool.tile([128, NT], BF16, tag="et")
                ats = ps_big.tile([128, 1024], F32, tag="pb")
                for i in range(DC):
                    for hf in range(HF):
                        nc.tensor.matmul(
                            ats[:, hf * 512:(hf + 1) * 512],
                            kT[i][:, c * 128:(c + 1) * 128],
                            qT[i][:, hf * 512:(hf + 1) * 512],
                            start=(i == 0), stop=(i == DC - 1))
                nc.scalar.activation(
                    out=et, in_=ats,
                    func=mybir.ActivationFunctionType.Exp, scale=ISQ)
                eT.append(et)

            z2 = [zpool.tile([128, 2 * DM], BF16, tag="z2", name=f"z{p}")
                  for p in range(RP)]
            zT = tchunk.tile([128, DC, NT], BF16, tag="tchunk", name="zT")
            # publish before the pair loop: after_pair callbacks for THIS
            # batch (last layer) read zTs[b]/zs[b] slices produced pairs
            # earlier; tile-slice dependencies handle readiness
            zs[b] = z2
            zTs[b] = zT
            for p in range(RP):
                r0 = 2 * p
                zps = ps_big.tile([128, 1024], F32, tag="pb")
                sp = [None, None]
                if l > 0:
                    sp = [ps_s.tile([128, 1], F32, tag="sp",
                                    name=f"sp{j}") for j in range(2)]
                for c in range(RT):
                    for j in range(2):
                        r = r0 + j
                        nc.tensor.matmul(
                            zps[:, j * 512:(j + 1) * 512],
                            eT[c][:, r * 128:(r + 1) * 128],
                            vsl(c), start=(c == 0), stop=(c == RT - 1))
                        if l > 0:
                            nc.tensor.matmul(
                                sp[j],
                                eT[c][:, r * 128:(r + 1) * 128], ones,
                                start=(c == 0), stop=(c == RT - 1))
                rs = None
                if l > 0:
                    rs = smallp.tile([128, 2], F32, tag="rs")
                    for j in range(2):
                        nc.vector.reciprocal(out=rs[:, j:j + 1], in_=sp[j])
                for j in range(2):
                    r = r0 + j
                    zsl = zps[:, j * 512:(j + 1) * 512]
                    osl = z2[p][:, j * 512:(j + 1) * 512]
                    zpre = tmpp.tile([128, DM], BF16, tag="zpre")
                    if affine_identity:
                        # fused drain: zpre = (Z * 1/s) + h (layer 0:
                        # + 0, 1/s folds into LN scale invariance)
                        nc.vector.scalar_tensor_tensor(
                            out=zpre, in0=zsl,
                            scalar=(rs[:, j:j + 1] if l > 0 else 1.0),
                            in1=(hsl(b, r) if l > 0 else warm),
                            op0=MULT, op1=ADD)
                        ln_a(zpre, osl)
                    else:
                        if l > 0:
                            tmp = tmpp.tile([128, DM], BF16, tag="tmp")
                            nc.scalar.activation(
                                out=tmp, in_=zsl,
                                func=mybir.ActivationFunctionType.Copy,
                                scale=rs[:, j:j + 1])
                            nc.vector.tensor_add(out=zpre, in0=tmp,
                                                 in1=hsl(b, r))
                        else:
                            nc.vector.tensor_copy(out=zpre, in_=zsl)
                        layernorm_r(zpre, osl, g1b, b1b)
                for j in range(2):
                    r = r0 + j
                    nc.sync.dma_start_transpose(
                        out=zT[:, :, r * 128:(r + 1) * 128],
                        in_=z2[p][:, (r % 2) * DM:(r % 2 + 1) * DM])
                slot()

        ffn_leftover = [None]   # callable emitting prev layer's FFN(b1) pair

        for l in range(3):
            lw = lws[l]
            if affine_identity:
                g1b = b1b = g2b = b2b = None
            else:
                g1b = load_ln("g1", l)
                b1b = load_ln("b1", l)
                g2b = load_ln("g2", l)
                b2b = load_ln("b2", l)
            bfb = None if bf_zero else load_ln("bf", l)

            def ffn_pair(b, p, g2t, nhT, lw=lw, bfb=bfb, g2b=g2b, b2b=b2b,
                         lpin=l):
                r0 = 2 * p
                fps = ps_big.tile([128, 1024], F32, tag="pb")
                for i in range(DC):
                    for j in range(2):
                        r = r0 + j
                        nc.tensor.matmul(
                            fps[:, j * 512:(j + 1) * 512],
                            zTs[b][:, i, r * 128:(r + 1) * 128],
                            lw["wf"][:, i, :],
                            start=(i == 0), stop=(i == DC - 1))
                if bfb is not None:
                    for j in range(2):
                        nc.vector.tensor_add(
                            out=fps[:, j * 512:(j + 1) * 512],
                            in0=fps[:, j * 512:(j + 1) * 512], in1=bfb)
                for j in range(2):
                    fsl = fps[:, j * 512:(j + 1) * 512]
                    osl = g2t[:, j * 512:(j + 1) * 512]
                    gpre = tmpp.tile([128, DM], BF16, tag="gpre")
                    if affine_identity:
                        # fused drain: gpre = max(f, 0) + z
                        nc.vector.scalar_tensor_tensor(
                            out=gpre, in0=fsl, scalar=0.0,
                            in1=zs[b][p][:, j * 512:(j + 1) * 512],
                            op0=MAX, op1=ADD)
                        ln_a(gpre, osl)
                    else:
                        f_r = tmpp.tile([128, DM], BF16, tag="fr")
                        nc.scalar.activation(
                            out=f_r, in_=fsl,
                            func=mybir.ActivationFunctionType.Relu)
                        nc.vector.tensor_add(
                            out=gpre, in0=f_r,
                            in1=zs[b][p][:, j * 512:(j + 1) * 512])
                        layernorm_r(gpre, osl, g2b, b2b)
                if lpin == 2:
                    # one [256,512] store per pair on the idle gpsimd
                    # SWDGE queue (keeps the tail off the HWDGE queues)
                    nc.gpsimd.dma_start(
                        out=d["out"][b * NT + r0 * 128:
                                     b * NT + (r0 + 2) * 128, :]
                        .rearrange("(j p) o -> p j o", j=2),
                        in_=g2t.rearrange("p (j o) -> p j o", j=2))
                else:
                    for j in range(2):
                        r = r0 + j
                        nc.sync.dma_start_transpose(
                            out=nhT[:, :, r * 128:(r + 1) * 128],
                            in_=g2t[:, j * 512:(j + 1) * 512])

            # phase 1 for batch 0 consumes the previous layer's leftover
            # FFN(b1) pairs (all their inputs are long since ready);
            # phase 1 for batch 1 consumes this layer's FFN(b0) pairs at
            # its V slots, so the h^T(b0) transposes complete well before
            # the next layer's QK chains.  FFN(b1) becomes the next
            # layer's leftover; for the last layer its first three pairs
            # ride in batch 1's Z slots and only pair 3 trails the kernel.
            phase1(l, 0, lw, g1b, b1b, work=ffn_leftover[0])
            ffn_leftover[0] = None

            def mk(b, p, g2t, nhT, ffn_pair=ffn_pair):
                return lambda: ffn_pair(b, p, g2t, nhT)

            if l < 2:
                nhT0 = tchunk.tile([128, DC, NT], BF16, tag="tchunk",
                                   name="hT0")
                ng0 = [gpool.tile([128, 2 * DM], BF16, tag="g2",
                                  name=f"g0_{p}") for p in range(RP)]
                phase1(l, 1, lw, g1b, b1b,
                       work=[mk(0, p, ng0[p], nhT0) for p in range(RP)])
                nhT1 = tchunk.tile([128, DC, NT], BF16, tag="tchunk",
                                   name="hT1")
                ng1 = [gpool.tile([128, 2 * DM], BF16, tag="g2",
                                  name=f"g1_{p}") for p in range(RP)]
                ffn_leftover[0] = [mk(1, p, ng1[p], nhT1) for p in range(RP)]
                hT[0], hT[1] = nhT0, nhT1
                h[0], h[1] = ng0, ng1
            else:
                gts = [gout.tile([128, 2 * DM], BF16, tag="gout",
                                 name=f"go_{p}") for p in range(RP)]
                gts1 = [gout.tile([128, 2 * DM], BF16, tag="gout",
                                  name=f"go1_{p}") for p in range(RP)]
                # V slots: FFN(b0) p0-p3; Z slot 0 skipped (pair 0's z^T
                # transposes enter the queue only at that slot); Z slots
                # 1-3: FFN(b1) p0-p2; FFN(b1) p3 is the kernel tail.
                phase1(l, 1, lw, g1b, b1b,
                       work=[mk(0, p, gts[p], None) for p in range(RP)]
                       + [None]
                       + [mk(1, p, gts1[p], None) for p in range(RP - 1)])
                ffn_pair(1, RP - 1, gts1[RP - 1], None)


def kernel(**inputs):
    x = np.asarray(inputs["x"], np.float32)          # [16, 1024, 64]
    bfdt = np.dtype(mybir.dt.np(BF16))

    def to_bf16(a):
        return np.ascontiguousarray(np.asarray(a, np.float32).astype(bfdt))

    shared = {
        "wq0": to_bf16(inputs["Wq0"]),
        "wk0": to_bf16(inputs["Wk0"]),
        "wv0": to_bf16(inputs["Wv0"]),
        "wqs": to_bf16(inputs["Wqs"]),
        "wks": to_bf16(inputs["Wks"]),
        "wvs": to_bf16(inputs["Wvs"]),
        "wf": to_bf16(inputs["Wf"]),
        "g1": np.ascontiguousarray(inputs["g1"], np.float32),
        "b1": np.ascontiguousarray(inputs["b1"], np.float32),
        "g2": np.ascontiguousarray(inputs["g2"], np.float32),
        "b2": np.ascontiguousarray(inputs["b2"], np.float32),
        "bf": np.ascontiguousarray(inputs["bf"], np.float32),
    }
    in_maps = []
    for i in range(NCORES):
        xt = to_bf16(
            np.concatenate([x[NB * i + b].T for b in range(NB)], axis=1))
        m = dict(shared)
        m["xt"] = xt
        in_maps.append(m)

    affine_identity = bool(
        np.all(shared["g1"] == 1) and np.all(shared["b1"] == 0)
        and np.all(shared["g2"] == 1) and np.all(shared["b2"] == 0))
    bf_zero = bool(np.all(shared["bf"] == 0))

    nc = bacc.Bacc()
    build_graph(nc, affine_identity=affine_identity, bf_zero=bf_zero)
    res = run_bass_kernel_spmd(nc, in_maps, list(range(NCORES)), trace=TRACE)
    if TRACE:
        print("exec_time_ns:", res.exec_time_ns, "mean:", res.mean_exec_time_ns)
        kernel.last_result = res

    y = np.empty((NCORES * NB, NT, DM), np.float32)
    for i in range(NCORES):
        o = np.asarray(res.results[i]["out"]).astype(np.float32)
        for b in range(NB):
            y[NB * i + b] = o[b * NT:(b + 1) * NT]
    return y


# revision 21
# speedup vs baseline: 1.4634x; 1.4634x over previous
"""AnomalyTransformer Trainium2 kernel.

3-layer transformer encoder (d=64 -> d_model=512, N=1024 tokens, B=16),
data-parallel over batch: 8 NeuronCores x 2 batches each, weights
replicated, no collectives.  The Gaussian-prior branch of the reference
is a dead computation (never touches the output) and is skipped.

Layout strategy per core (per batch, N=1024 tokens):
  - Input is pre-transposed AND bf16-converted on host: xt [64, 2048].
  - All matmuls run in bf16 (fp32 PSUM accumulation).
  - QKV projections produce Q^T, K^T (dm-chunk partition, token free)
    and V row-major, all bf16.
  - Attention scores are computed directly TRANSPOSED: A^T[col, row] so
    that exp(A^T) tiles are immediately usable as matmul lhsT for
    Z = softmax(A) @ V without any transposes.
  - Softmax uses no max-subtraction (logits empirically bounded ~15);
    normalization is folded into the residual via one fused DVE
    scalar_tensor_tensor: zpre = (Z * 1/s) + h, which also emits the
    true row-sum (accum_out) for the LN that follows.  The FFN drain is
    the same shape: gpre = max(f, 0) + z in one fused op.
  - PSUM tiles are [128, 1024] spanning two banks; matmul chains fill
    the two 512-wide halves (each within one bank).  The fused drain op
    reads PSUM at the head of each drain chain, so banks recycle fast.
  - QK projection chains run hf-outer (token-half outer loop) so the
    first chains of layer l+1 depend only on the first half of the h^T
    transposes; Q/K/V PSUM drains alternate between DVE and ACT.
  - LayerNorm alternates two engine assignments by row parity to keep
    DVE and ACT equally loaded: the A-form computes stats with
    bn_stats/bn_aggr on the bf16 zpre (DVE), the B-form squares on ACT
    with a free fp32 accumulator and only runs tiny [128,1] chains on
    the DVE.  Both end with sqrt on ACT -> reciprocal + normalize on
    DVE (one cross-engine hop each).
  - DMA is spread over the three queues: weights + inputs + xbar
    transposes (z^T/g^T) on the sync HWDGE queue (weight prefetch must
    NOT sit on the scalar queue: its serialized DMA_DIRECT2Ds
    head-of-line block the ACT FIFO for ~10us), output stores on the
    otherwise-idle gpsimd SWDGE queue, merged to one [256,512] store
    per row-pair.  z^T transposes are enqueued per row-pair right after
    the producing LN so no queue ever faces an 8-transpose burst.
  - Software pipelining across batches: each layer's FFN for batch 0 is
    interleaved into batch 1's Z phase at row-pair granularity, and the
    FFN for batch 1 is interleaved into the NEXT layer's batch-0 QKV
    phase.  Drain chains therefore always overlap a long stretch of
    independent matmuls, and only the last layer's FFN(b1) remains as
    the kernel tail.
  - A short burst of dummy matmuls on a zeroed tile runs while the
    input DMA is in flight: the PE's HAM clock-gate needs ~3.4us of
    sustained activity to lift the default 4/8 throttle, and the burst
    buys that warm-up during otherwise-idle head time.
  - When the affine params are not identity the general bn_stats path
    with explicit affine ops is used instead (correct, less tuned).
"""

import numpy as np

import concourse.bass as bass
import concourse.mybir as mybir
import concourse.tile as tile
from concourse import bacc
from concourse.bass_utils import run_bass_kernel_spmd

F32 = mybir.dt.float32
BF16 = mybir.dt.bfloat16
TRACE = False

D0 = 64      # input feature dim
DM = 512     # d_model
NT = 1024    # tokens per batch
NB = 2       # batches per core
NCORES = 8
DC = DM // 128   # 4 dm chunks
RT = NT // 128   # 8 token tiles per batch
RP = RT // 2     # 4 row pairs per batch
HF = NT // 512   # 2 moving-operand halves
ISQ = 1.0 / float(np.sqrt(DM))
EPS = 1e-5
NWARM = 6        # dummy matmuls to lift the HAM clock throttle


def build_graph(nc, affine_identity=False, bf_zero=False):
    T = NB * NT

    d = {}
    d["xt"] = nc.declare_dram_parameter("xt", [D0, T], BF16, isOutput=False)
    for nm in ("wq0", "wk0", "wv0"):
        d[nm] = nc.declare_dram_parameter(nm, [D0, DM], BF16, isOutput=False)
    for nm, L in (("wqs", 2), ("wks", 2), ("wvs", 2), ("wf", 3)):
        d[nm] = nc.declare_dram_parameter(nm, [L, DM, DM], BF16, isOutput=False)
    for nm in ("g1", "b1", "g2", "b2", "bf"):
        d[nm] = nc.declare_dram_parameter(nm, [3, DM], F32, isOutput=False)
    # bf16 output: halves the store traffic in the kernel tail; the host
    # upcasts to f32 (adds ~0.2% element error, well inside the budget)
    d["out"] = nc.declare_dram_parameter("out", [T, DM], BF16, isOutput=True)

    with tile.TileContext(nc) as tc:
        _build_tc(tc, nc, d, affine_identity, bf_zero)
    nc.compile()
    return nc


def _build_tc(tc, nc, d, affine_identity=False, bf_zero=False):
    from contextlib import ExitStack
    ctx = ExitStack()
    with ctx:
        const = ctx.enter_context(tc.tile_pool(name="const", bufs=1))
        wpool = ctx.enter_context(tc.tile_pool(name="wpool", bufs=9))
        lnpool = ctx.enter_context(tc.tile_pool(name="lnpool", bufs=8))
        qkpool = ctx.enter_context(tc.tile_pool(name="qkpool", bufs=8))
        vpool = ctx.enter_context(tc.tile_pool(name="vpool", bufs=5))
        epool = ctx.enter_context(tc.tile_pool(name="epool", bufs=8))
        zpool = ctx.enter_context(tc.tile_pool(name="zpool", bufs=8))
        gpool = ctx.enter_context(tc.tile_pool(name="gpool", bufs=10))
        tchunk = ctx.enter_context(tc.tile_pool(name="tchunk", bufs=4))
        xpool = ctx.enter_context(tc.tile_pool(name="xpool", bufs=2))
        smallp = ctx.enter_context(tc.tile_pool(name="smallp", bufs=12))
        tmpp = ctx.enter_context(tc.tile_pool(name="tmpp", bufs=7))
        sqpool = ctx.enter_context(tc.tile_pool(name="sqpool", bufs=3))
        fpool = ctx.enter_context(tc.tile_pool(name="fpool", bufs=3))
        gout = ctx.enter_context(tc.tile_pool(name="gout", bufs=4))
        ps_big = ctx.enter_context(tc.tile_pool(name="ps_big", bufs=3,
                                                space="PSUM"))
        # two bufs: the two row-sum chains of a pair must live in separate
        # banks (a chain's first matmul clears the whole bank's has_written)
        ps_s = ctx.enter_context(tc.tile_pool(name="ps_s", bufs=2,
                                              space="PSUM"))

        # input + ALL weights on the sync HWDGE queue: the first QK chains
        # need only xt half 0 + wq0; the big weight loads land by ~25us
        # (first needed ~100us).  Keeping them OFF the scalar queue matters:
        # serialized DMA_DIRECT2Ds there head-of-line block the ACT FIFO.
        xts = [xpool.tile([D0, NT], BF16, tag="xt", name=f"xt{b}")
               for b in range(NB)]
        nc.sync.dma_start(out=xts[0][:, 0:512], in_=d["xt"][:, 0:512])
        w0 = {}
        for name in ("wq0", "wk0"):
            t = const.tile([D0, DM], BF16, tag=name)
            nc.sync.dma_start(out=t, in_=d[name][:])
            w0[name] = t
        nc.sync.dma_start(out=xts[0][:, 512:1024], in_=d["xt"][:, 512:1024])
        t = const.tile([D0, DM], BF16, tag="wv0")
        nc.sync.dma_start(out=t, in_=d["wv0"][:])
        w0["wv0"] = t
        for hf in range(HF):
            nc.sync.dma_start(
                out=xts[1][:, hf * 512:(hf + 1) * 512],
                in_=d["xt"][:, NT + hf * 512:NT + (hf + 1) * 512])

        ones = const.tile([128, 1], BF16)
        nc.vector.memset(ones, 1.0)
        eps_t = const.tile([128, 1], F32)
        nc.vector.memset(eps_t, EPS)

        # zeroed [128,512] tile: HAM warm-up operand + the "+0" second
        # input of the L0 fused drain (no residual in layer 0)
        warm = const.tile([128, 512], BF16, tag="warm")
        nc.vector.memset(warm, 0.0)
        wps = ps_big.tile([128, 1024], F32, tag="pb")
        for i in range(NWARM):
            nc.tensor.matmul(wps[:, 0:512], warm[:, 0:128], warm,
                             start=True, stop=True)

        def load_w(key, idx):
            t = wpool.tile([128, DC, DM], BF16, tag="W")
            nc.sync.dma_start(
                out=t, in_=d[key][idx].rearrange("(c p) o -> p c o", p=128))
            return t

        lws = []
        for l in range(3):
            lwd = {}
            if l > 0:
                for nm, key in (("wq", "wqs"), ("wk", "wks"), ("wv", "wvs")):
                    lwd[nm] = load_w(key, l - 1)
            lwd["wf"] = load_w("wf", l)
            lws.append(lwd)

        def load_ln(name, l):
            t = lnpool.tile([128, DM], F32, tag="ln")
            nc.scalar.dma_start(
                out=t, in_=d[name][l].unsqueeze(0).to_broadcast((128, DM)))
            return t

        IDM = 1.0 / DM
        MULT = mybir.AluOpType.mult
        ADD = mybir.AluOpType.add
        SUB = mybir.AluOpType.subtract
        MAX = mybir.AluOpType.max

        def layernorm_r(zpre, out_ap, gb, bb):
            """General-path LN (non-identity affine): bn_stats on DVE."""
            stats = smallp.tile([128, 6], F32, tag="stats")
            mv = smallp.tile([128, 2], F32, tag="mv")
            nc.vector.bn_stats(out=stats, in_=zpre)
            nc.vector.bn_aggr(out=mv, in_=stats)
            stdv = smallp.tile([128, 1], F32, tag="stdv")
            nc.scalar.activation(out=stdv, in_=mv[:, 1:2],
                                 func=mybir.ActivationFunctionType.Sqrt,
                                 bias=eps_t, scale=1.0)
            rstd = smallp.tile([128, 1], F32, tag="rstd")
            nc.vector.reciprocal(out=rstd, in_=stdv)
            nc.vector.tensor_scalar(
                out=out_ap, in0=zpre, scalar1=mv[:, 0:1], scalar2=rstd,
                op0=SUB, op1=MULT)
            if gb is not None:
                nc.vector.tensor_mul(out=out_ap, in0=out_ap, in1=gb)
                nc.vector.tensor_add(out=out_ap, in0=out_ap, in1=bb)

        POW = mybir.AluOpType.pow

        def ln_a(zpre, out_ap):
            """LN: bn_stats/bn_aggr (DVE) -> sqrt (ACT; unavoidable --
            Sqrt and Exp live in different ACT spline-table sets and the
            set loader is per-function-static, so the ~2 set switches per
            phase are the floor) -> reciprocal + normalize (DVE)."""
            stats = smallp.tile([128, 6], F32, tag="stats")
            mv = smallp.tile([128, 2], F32, tag="mv")
            nc.vector.bn_stats(out=stats, in_=zpre)
            nc.vector.bn_aggr(out=mv, in_=stats)
            stdv = smallp.tile([128, 1], F32, tag="stdv")
            nc.scalar.activation(out=stdv, in_=mv[:, 1:2],
                                 func=mybir.ActivationFunctionType.Sqrt,
                                 bias=eps_t, scale=1.0)
            rstd = smallp.tile([128, 1], F32, tag="rstd")
            nc.vector.reciprocal(out=rstd, in_=stdv)
            nc.vector.tensor_scalar(
                out=out_ap, in0=zpre, scalar1=mv[:, 0:1], scalar2=rstd,
                op0=SUB, op1=MULT)

        hT = [None] * NB    # [128, DC, NT] bf16 per batch (transposed h)
        h = [None] * NB     # RP pair tiles [128, 1024] bf16 per batch
        zs = [None] * NB
        zTs = [None] * NB

        def hsl(b, r):
            return h[b][r // 2][:, (r % 2) * DM:(r % 2 + 1) * DM]

        def phase1(l, b, lw, g1b, b1b, work=None):
            """QKV + scores/exp + Z + LN1 (+ per-pair z^T transposes) for
            one batch.  `work` is an ordered list of pending FFN-pair
            thunks (or None entries to skip a slot), consumed one per
            V-pair and one per Z-pair: drain bursts always overlap a
            stretch of independent matmuls, and cross-phase inputs (z^T /
            h^T transposes) are produced long before they are needed."""
            widx = [0]

            def slot():
                if work is not None and widx[0] < len(work):
                    thunk = work[widx[0]]
                    widx[0] += 1
                    if thunk is not None:
                        thunk()
            qT = [qkpool.tile([128, NT], BF16, tag="qk", name=f"qT{o}")
                  for o in range(DC)]
            kT = [qkpool.tile([128, NT], BF16, tag="qk", name=f"kT{o}")
                  for o in range(DC)]
            v2 = [vpool.tile([128, 2 * DM], BF16, tag="v2", name=f"v{p}")
                  for p in range(RP)]

            def vsl(c):
                return v2[c // 2][:, (c % 2) * DM:(c % 2 + 1) * DM]

            # QK chains: one [128,1024] PSUM tile per dm-chunk (both token
            # halves); full-tile drains alternate DVE/ACT.  (The h^T
            # transposes these chains depend on complete with lots of
            # slack now that the producing FFN is interleaved a phase
            # earlier, so no hf-ordering tricks are needed.)
            for dst, wname in ((qT, "wq"), (kT, "wk")):
                for o in range(DC):
                    ps = ps_big.tile([128, 1024], F32, tag="pb")
                    for hf in range(HF):
                        psl = ps[:, hf * 512:(hf + 1) * 512]
                        if l == 0:
                            w0n = "wq0" if wname == "wq" else "wk0"
                            nc.tensor.matmul(
                                psl,
                                w0[w0n][:, o * 128:(o + 1) * 128],
                                xts[b][:, hf * 512:(hf + 1) * 512],
                                start=True, stop=True)
                        else:
                            for i in range(DC):
                                nc.tensor.matmul(
                                    psl,
                                    lw[wname][:, i, o * 128:(o + 1) * 128],
                                    hT[b][:, i, hf * 512:(hf + 1) * 512],
                                    start=(i == 0), stop=(i == DC - 1))
                    nc.scalar.copy(out=dst[o], in_=ps)

            for p in range(RP):
                ps = ps_big.tile([128, 1024], F32, tag="pb")
                if l == 0:
                    for j in range(2):
                        r = 2 * p + j
                        nc.tensor.matmul(
                            ps[:, j * 512:(j + 1) * 512],
                            xts[b][:, r * 128:(r + 1) * 128], w0["wv0"],
                            start=True, stop=True)
                else:
                    for i in range(DC):
                        for j in range(2):
                            r = 2 * p + j
                            nc.tensor.matmul(
                                ps[:, j * 512:(j + 1) * 512],
                                hT[b][:, i, r * 128:(r + 1) * 128],
                                lw["wv"][:, i, :],
                                start=(i == 0), stop=(i == DC - 1))
                nc.scalar.copy(out=v2[p], in_=ps)
                slot()

            eT = []
            for c in range(RT):
                et = epool.tile([128, NT], BF16, tag="et")
                ats = ps_big.tile([128, 1024], F32, tag="pb")
                for i in range(DC):
                    for hf in range(HF):
                        nc.tensor.matmul(
                            ats[:, hf * 512:(hf + 1) * 512],
                            kT[i][:, c * 128:(c + 1) * 128],
                            qT[i][:, hf * 512:(hf + 1) * 512],
                            start=(i == 0), stop=(i == DC - 1))
                nc.scalar.activation(
                    out=et, in_=ats,
                    func=mybir.ActivationFunctionType.Exp, scale=ISQ)
                eT.append(et)

            z2 = [zpool.tile([128, 2 * DM], BF16, tag="z2", name=f"z{p}")
                  for p in range(RP)]
            zT = tchunk.tile([128, DC, NT], BF16, tag="tchunk", name="zT")
            # publish before the pair loop: after_pair callbacks for THIS
            # batch (last layer) read zTs[b]/zs[b] slices produced pairs
            # earlier; tile-slice dependencies handle readiness
            zs[b] = z2
            zTs[b] = zT
            for p in range(RP):
                r0 = 2 * p
                zps = ps_big.tile([128, 1024], F32, tag="pb")
                sp = [None, None]
                if l > 0:
                    sp = [ps_s.tile([128, 1], F32, tag="sp",
                                    name=f"sp{j}") for j in range(2)]
                for c in range(RT):
                    for j in range(2):
                        r = r0 + j
                        nc.tensor.matmul(
                            zps[:, j * 512:(j + 1) * 512],
                            eT[c][:, r * 128:(r + 1) * 128],
                            vsl(c), start=(c == 0), stop=(c == RT - 1))
                        if l > 0:
                            nc.tensor.matmul(
                                sp[j],
                                eT[c][:, r * 128:(r + 1) * 128], ones,
                                start=(c == 0), stop=(c == RT - 1))
                rs = None
                if l > 0:
                    rs = smallp.tile([128, 2], F32, tag="rs")
                    for j in range(2):
                        nc.vector.reciprocal(out=rs[:, j:j + 1], in_=sp[j])
                for j in range(2):
                    r = r0 + j
                    zsl = zps[:, j * 512:(j + 1) * 512]
                    osl = z2[p][:, j * 512:(j + 1) * 512]
                    zpre = tmpp.tile([128, DM], BF16, tag="zpre")
                    if affine_identity:
                        # fused drain: zpre = (Z * 1/s) + h (layer 0:
                        # + 0, 1/s folds into LN scale invariance)
                        nc.vector.scalar_tensor_tensor(
                            out=zpre, in0=zsl,
                            scalar=(rs[:, j:j + 1] if l > 0 else 1.0),
                            in1=(hsl(b, r) if l > 0 else warm),
                            op0=MULT, op1=ADD)
                        ln_a(zpre, osl)
                    else:
                        if l > 0:
                            tmp = tmpp.tile([128, DM], BF16, tag="tmp")
                            nc.scalar.activation(
                                out=tmp, in_=zsl,
                                func=mybir.ActivationFunctionType.Copy,
                                scale=rs[:, j:j + 1])
                            nc.vector.tensor_add(out=zpre, in0=tmp,
                                                 in1=hsl(b, r))
                        else:
                            nc.vector.tensor_copy(out=zpre, in_=zsl)
                        layernorm_r(zpre, osl, g1b, b1b)
                for j in range(2):
                    r = r0 + j
                    nc.sync.dma_start_transpose(
                        out=zT[:, :, r * 128:(r + 1) * 128],
                        in_=z2[p][:, (r % 2) * DM:(r % 2 + 1) * DM])
                slot()

        ffn_leftover = [None]   # callable emitting prev layer's FFN(b1) pair

        for l in range(3):
            lw = lws[l]
            if affine_identity:
                g1b = b1b = g2b = b2b = None
            else:
                g1b = load_ln("g1", l)
                b1b = load_ln("b1", l)
                g2b = load_ln("g2", l)
                b2b = load_ln("b2", l)
            bfb = None if bf_zero else load_ln("bf", l)

            def ffn_pair(b, p, g2t, nhT, lw=lw, bfb=bfb, g2b=g2b, b2b=b2b,
                         lpin=l):
                r0 = 2 * p
                fps = ps_big.tile([128, 1024], F32, tag="pb")
                for i in range(DC):
                    for j in range(2):
                        r = r0 + j
                        nc.tensor.matmul(
                            fps[:, j * 512:(j + 1) * 512],
                            zTs[b][:, i, r * 128:(r + 1) * 128],
                            lw["wf"][:, i, :],
                            start=(i == 0), stop=(i == DC - 1))
                if bfb is not None:
                    for j in range(2):
                        nc.vector.tensor_add(
                            out=fps[:, j * 512:(j + 1) * 512],
                            in0=fps[:, j * 512:(j + 1) * 512], in1=bfb)
                for j in range(2):
                    fsl = fps[:, j * 512:(j + 1) * 512]
                    osl = g2t[:, j * 512:(j + 1) * 512]
                    gpre = tmpp.tile([128, DM], BF16, tag="gpre")
                    if affine_identity:
                        # fused drain: gpre = max(f, 0) + z
                        nc.vector.scalar_tensor_tensor(
                            out=gpre, in0=fsl, scalar=0.0,
                            in1=zs[b][p][:, j * 512:(j + 1) * 512],
                            op0=MAX, op1=ADD)
                        ln_a(gpre, osl)
                    else:
                        f_r = tmpp.tile([128, DM], BF16, tag="fr")
                        nc.scalar.activation(
                            out=f_r, in_=fsl,
                            func=mybir.ActivationFunctionType.Relu)
                        nc.vector.tensor_add(
                            out=gpre, in0=f_r,
                            in1=zs[b][p][:, j * 512:(j + 1) * 512])
                        layernorm_r(gpre, osl, g2b, b2b)
                if lpin == 2:
                    # one [256,512] store per pair on the idle gpsimd
                    # SWDGE queue (keeps the tail off the HWDGE queues)
                    nc.gpsimd.dma_start(
                        out=d["out"][b * NT + r0 * 128:
                                     b * NT + (r0 + 2) * 128, :]
                        .rearrange("(j p) o -> p j o", j=2),
                        in_=g2t.rearrange("p (j o) -> p j o", j=2))
                else:
                    for j in range(2):
                        r = r0 + j
                        nc.sync.dma_start_transpose(
                            out=nhT[:, :, r * 128:(r + 1) * 128],
                            in_=g2t[:, j * 512:(j + 1) * 512])

            # phase 1 for batch 0 consumes the previous layer's leftover
            # FFN(b1) pairs (all their inputs are long since ready);
            # phase 1 for batch 1 consumes this layer's FFN(b0) pairs at
            # its V slots, so the h^T(b0) transposes complete well before
            # the next layer's QK chains.  FFN(b1) becomes the next
            # layer's leftover; for the last layer its first three pairs
            # ride in batch 1's Z slots and only pair 3 trails the kernel.
            phase1(l, 0, lw, g1b, b1b, work=ffn_leftover[0])
            ffn_leftover[0] = None

            def mk(b, p, g2t, nhT, ffn_pair=ffn_pair):
                return lambda: ffn_pair(b, p, g2t, nhT)

            if l < 2:
                nhT0 = tchunk.tile([128, DC, NT], BF16, tag="tchunk",
                                   name="hT0")
                ng0 = [gpool.tile([128, 2 * DM], BF16, tag="g2",
                                  name=f"g0_{p}") for p in range(RP)]
                phase1(l, 1, lw, g1b, b1b,
                       work=[mk(0, p, ng0[p], nhT0) for p in range(RP)])
                nhT1 = tchunk.tile([128, DC, NT], BF16, tag="tchunk",
                                   name="hT1")
                ng1 = [gpool.tile([128, 2 * DM], BF16, tag="g2",
                                  name=f"g1_{p}") for p in range(RP)]
                ffn_leftover[0] = [mk(1, p, ng1[p], nhT1) for p in range(RP)]
                hT[0], hT[1] = nhT0, nhT1
                h[0], h[1] = ng0, ng1
            else:
                gts = [gout.tile([128, 2 * DM], BF16, tag="gout",
                                 name=f"go_{p}") for p in range(RP)]
                gts1 = [gout.tile([128, 2 * DM], BF16, tag="gout",
                                  name=f"go1_{p}") for p in range(RP)]
                # V slots: FFN(b0) p0-p3; Z slot 0 skipped (pair 0's z^T
                # transposes enter the queue only at that slot); Z slots
                # 1-3: FFN(b1) p0-p2; FFN(b1) p3 is the kernel tail.
                phase1(l, 1, lw, g1b, b1b,
                       work=[mk(0, p, gts[p], None) for p in range(RP)]
                       + [None]
                       + [mk(1, p, gts1[p], None) for p in range(RP - 1)])
                ffn_pair(1, RP - 1, gts1[RP - 1], None)


def kernel(**inputs):
    x = np.asarray(inputs["x"], np.float32)          # [16, 1024, 64]
    bfdt = np.dtype(mybir.dt.np(BF16))

    def to_bf16(a):
        return np.ascontiguousarray(np.asarray(a, np.float32).astype(bfdt))

    shared = {
        "wq0": to_bf16(inputs["Wq0"]),
        "wk0": to_bf16(inputs["Wk0"]),
        "wv0": to_bf16(inputs["Wv0"]),
        "wqs": to_bf16(inputs["Wqs"]),
        "wks": to_bf16(inputs["Wks"]),
        "wvs": to_bf16(inputs["Wvs"]),
        "wf": to_bf16(inputs["Wf"]),
        "g1": np.ascontiguousarray(inputs["g1"], np.float32),
        "b1": np.ascontiguousarray(inputs["b1"], np.float32),
        "g2": np.ascontiguousarray(inputs["g2"], np.float32),
        "b2": np.ascontiguousarray(inputs["b2"], np.float32),
        "bf": np.ascontiguousarray(inputs["bf"], np.float32),
    }
    in_maps = []
    for i in range(NCORES):
        xt = to_bf16(
            np.concatenate([x[NB * i + b].T for b in range(NB)], axis=1))
        m = dict(shared)
        m["xt"] = xt
        in_maps.append(m)

    affine_identity = bool(
        np.all(shared["g1"] == 1) and np.all(shared["b1"] == 0)
        and np.all(shared["g2"] == 1) and np.all(shared["b2"] == 0))
    bf_zero = bool(np.all(shared["bf"] == 0))

    nc = bacc.Bacc()
    build_graph(nc, affine_identity=affine_identity, bf_zero=bf_zero)
    res = run_bass_kernel_spmd(nc, in_maps, list(range(NCORES)), trace=TRACE)
    if TRACE:
        print("exec_time_ns:", res.exec_time_ns, "mean:", res.mean_exec_time_ns)
        kernel.last_result = res

    y = np.empty((NCORES * NB, NT, DM), np.float32)
    for i in range(NCORES):
        o = np.asarray(res.results[i]["out"]).astype(np.float32)
        for b in range(NB):
            y[NB * i + b] = o[b * NT:(b + 1) * NT]
    return y


# revision 25
# speedup vs baseline: 1.7455x; 1.1928x over previous
"""AnomalyTransformer Trainium2 kernel.

3-layer transformer encoder (d=64 -> d_model=512, N=1024 tokens, B=16),
data-parallel over batch: 8 NeuronCores x 2 batches each, weights
replicated, no collectives.  The Gaussian-prior branch of the reference
is a dead computation (never touches the output) and is skipped.

Layout strategy per core (per batch, N=1024 tokens):
  - Input is pre-transposed AND bf16-converted on host: xt [64, 2048].
  - All matmuls run in bf16 (fp32 PSUM accumulation).
  - QKV projections produce Q^T, K^T (dm-chunk partition, token free)
    and V row-major, all bf16.
  - Attention scores are computed directly TRANSPOSED: A^T[col, row] so
    that exp(A^T) tiles are immediately usable as matmul lhsT for
    Z = softmax(A) @ V without any transposes.
  - Softmax uses no max-subtraction (logits empirically bounded ~15);
    normalization is folded into the residual via one fused DVE
    scalar_tensor_tensor: zpre = (Z * 1/s) + h.  The FFN drain is the
    same shape: gpre = max(f, 0) + z in one fused op.  These fused ops
    read PSUM at the head of each drain chain, so banks recycle fast.
  - PSUM tiles are [128, 1024] spanning two banks; matmul chains fill
    the two 512-wide halves (each within one bank).
  - Engine split: ACT owns throughput drains (exp, Q/K/V PSUM copies)
    plus the LN sqrt; DVE owns the fused residual drains, bn_stats/
    bn_aggr, reciprocal and the final normalize.  The LN sqrt must stay
    on ACT (no DVE sqrt/pow) and costs ~2 ACT spline-table switches per
    phase against Exp's table set -- the measured floor.  The kernel
    tail's last FFN pair instead uses an ACT Square accumulator LN so
    the trailing chain leans on the otherwise-idle ACT.
  - DMA is spread over the three queues: weights + inputs + xbar
    transposes (z^T/g^T) on the sync HWDGE queue (weight prefetch must
    NOT sit on the scalar queue: its serialized DMA_DIRECT2Ds
    head-of-line block the ACT FIFO for ~10us), output stores on the
    otherwise-idle gpsimd SWDGE queue, merged to one [256,512] store
    per row-pair.  z^T transposes are enqueued per row-pair right after
    the producing LN so no queue ever faces an 8-transpose burst.
  - Software pipelining across batches: each layer's FFN for batch 0 is
    interleaved into batch 1's Z phase at row-pair granularity, and the
    FFN for batch 1 is interleaved into the NEXT layer's batch-0 QKV
    phase.  Drain chains therefore always overlap a long stretch of
    independent matmuls, and only the last layer's FFN(b1) remains as
    the kernel tail.
  - A short burst of dummy matmuls on a zeroed tile runs while the
    input DMA is in flight: the PE's HAM clock-gate needs ~3.4us of
    sustained activity to lift the default 4/8 throttle, and the burst
    buys that warm-up during otherwise-idle head time.
  - When the affine params are not identity the general bn_stats path
    with explicit affine ops is used instead (correct, less tuned).
"""

import numpy as np

import concourse.bass as bass
import concourse.mybir as mybir
import concourse.tile as tile
from concourse import bacc
from concourse.bass_utils import run_bass_kernel_spmd

F32 = mybir.dt.float32
BF16 = mybir.dt.bfloat16
TRACE = False

D0 = 64      # input feature dim
DM = 512     # d_model
NT = 1024    # tokens per batch
NB = 2       # batches per core
NCORES = 8
DC = DM // 128   # 4 dm chunks
RT = NT // 128   # 8 token tiles per batch
RP = RT // 2     # 4 row pairs per batch
HF = NT // 512   # 2 moving-operand halves
ISQ = 1.0 / float(np.sqrt(DM))
EPS = 1e-5
NWARM = 6        # dummy matmuls to lift the HAM clock throttle


def build_graph(nc, affine_identity=False, bf_zero=False):
    T = NB * NT

    d = {}
    d["xt"] = nc.declare_dram_parameter("xt", [D0, T], BF16, isOutput=False)
    for nm in ("wq0", "wk0", "wv0"):
        d[nm] = nc.declare_dram_parameter(nm, [D0, DM], BF16, isOutput=False)
    for nm, L in (("wqs", 2), ("wks", 2), ("wvs", 2), ("wf", 3)):
        d[nm] = nc.declare_dram_parameter(nm, [L, DM, DM], BF16, isOutput=False)
    for nm in ("g1", "b1", "g2", "b2", "bf"):
        d[nm] = nc.declare_dram_parameter(nm, [3, DM], F32, isOutput=False)
    # bf16 output: halves the store traffic in the kernel tail; the host
    # upcasts to f32 (adds ~0.2% element error, well inside the budget)
    d["out"] = nc.declare_dram_parameter("out", [T, DM], BF16, isOutput=True)

    with tile.TileContext(nc) as tc:
        _build_tc(tc, nc, d, affine_identity, bf_zero)
    nc.compile()
    return nc


def _build_tc(tc, nc, d, affine_identity=False, bf_zero=False):
    from contextlib import ExitStack
    ctx = ExitStack()
    with ctx:
        const = ctx.enter_context(tc.tile_pool(name="const", bufs=1))
        wpool = ctx.enter_context(tc.tile_pool(name="wpool", bufs=9))
        lnpool = ctx.enter_context(tc.tile_pool(name="lnpool", bufs=8))
        qkpool = ctx.enter_context(tc.tile_pool(name="qkpool", bufs=8))
        vpool = ctx.enter_context(tc.tile_pool(name="vpool", bufs=5))
        epool = ctx.enter_context(tc.tile_pool(name="epool", bufs=8))
        zpool = ctx.enter_context(tc.tile_pool(name="zpool", bufs=8))
        gpool = ctx.enter_context(tc.tile_pool(name="gpool", bufs=10))
        tchunk = ctx.enter_context(tc.tile_pool(name="tchunk", bufs=4))
        xpool = ctx.enter_context(tc.tile_pool(name="xpool", bufs=2))
        smallp = ctx.enter_context(tc.tile_pool(name="smallp", bufs=12))
        tmpp = ctx.enter_context(tc.tile_pool(name="tmpp", bufs=7))
        sqpool = ctx.enter_context(tc.tile_pool(name="sqpool", bufs=3))
        fpool = ctx.enter_context(tc.tile_pool(name="fpool", bufs=3))
        gout = ctx.enter_context(tc.tile_pool(name="gout", bufs=4))
        ps_big = ctx.enter_context(tc.tile_pool(name="ps_big", bufs=3,
                                                space="PSUM"))
        # two bufs: the two row-sum chains of a pair must live in separate
        # banks (a chain's first matmul clears the whole bank's has_written)
        ps_s = ctx.enter_context(tc.tile_pool(name="ps_s", bufs=2,
                                              space="PSUM"))

        # input + ALL weights on the sync HWDGE queue: the first QK chains
        # need only xt half 0 + wq0; the big weight loads land by ~25us
        # (first needed ~100us).  Keeping them OFF the scalar queue matters:
        # serialized DMA_DIRECT2Ds there head-of-line block the ACT FIFO.
        xts = [xpool.tile([D0, NT], BF16, tag="xt", name=f"xt{b}")
               for b in range(NB)]
        nc.sync.dma_start(out=xts[0][:, 0:512], in_=d["xt"][:, 0:512])
        w0 = {}
        for name in ("wq0", "wk0"):
            t = const.tile([D0, DM], BF16, tag=name)
            nc.sync.dma_start(out=t, in_=d[name][:])
            w0[name] = t
        nc.sync.dma_start(out=xts[0][:, 512:1024], in_=d["xt"][:, 512:1024])
        t = const.tile([D0, DM], BF16, tag="wv0")
        nc.sync.dma_start(out=t, in_=d["wv0"][:])
        w0["wv0"] = t
        for hf in range(HF):
            nc.sync.dma_start(
                out=xts[1][:, hf * 512:(hf + 1) * 512],
                in_=d["xt"][:, NT + hf * 512:NT + (hf + 1) * 512])

        ones = const.tile([128, 1], BF16)
        nc.vector.memset(ones, 1.0)
        eps_t = const.tile([128, 1], F32)
        nc.vector.memset(eps_t, EPS)

        # zeroed [128,512] tile: HAM warm-up operand + the "+0" second
        # input of the L0 fused drain (no residual in layer 0)
        warm = const.tile([128, 512], BF16, tag="warm")
        nc.vector.memset(warm, 0.0)
        wps = ps_big.tile([128, 1024], F32, tag="pb")
        for i in range(NWARM):
            nc.tensor.matmul(wps[:, 0:512], warm[:, 0:128], warm,
                             start=True, stop=True)

        def load_w(key, idx):
            t = wpool.tile([128, DC, DM], BF16, tag="W")
            nc.sync.dma_start(
                out=t, in_=d[key][idx].rearrange("(c p) o -> p c o", p=128))
            return t

        lws = []
        for l in range(3):
            lwd = {}
            if l > 0:
                for nm, key in (("wq", "wqs"), ("wk", "wks"), ("wv", "wvs")):
                    lwd[nm] = load_w(key, l - 1)
            lwd["wf"] = load_w("wf", l)
            lws.append(lwd)

        def load_ln(name, l):
            t = lnpool.tile([128, DM], F32, tag="ln")
            nc.scalar.dma_start(
                out=t, in_=d[name][l].unsqueeze(0).to_broadcast((128, DM)))
            return t

        IDM = 1.0 / DM
        MULT = mybir.AluOpType.mult
        ADD = mybir.AluOpType.add
        SUB = mybir.AluOpType.subtract
        MAX = mybir.AluOpType.max

        def layernorm_r(zpre, out_ap, gb, bb):
            """General-path LN (non-identity affine): bn_stats on DVE."""
            stats = smallp.tile([128, 6], F32, tag="stats")
            mv = smallp.tile([128, 2], F32, tag="mv")
            nc.vector.bn_stats(out=stats, in_=zpre)
            nc.vector.bn_aggr(out=mv, in_=stats)
            stdv = smallp.tile([128, 1], F32, tag="stdv")
            nc.scalar.activation(out=stdv, in_=mv[:, 1:2],
                                 func=mybir.ActivationFunctionType.Sqrt,
                                 bias=eps_t, scale=1.0)
            rstd = smallp.tile([128, 1], F32, tag="rstd")
            nc.vector.reciprocal(out=rstd, in_=stdv)
            nc.vector.tensor_scalar(
                out=out_ap, in0=zpre, scalar1=mv[:, 0:1], scalar2=rstd,
                op0=SUB, op1=MULT)
            if gb is not None:
                nc.vector.tensor_mul(out=out_ap, in0=out_ap, in1=gb)
                nc.vector.tensor_add(out=out_ap, in0=out_ap, in1=bb)

        POW = mybir.AluOpType.pow

        def ln_a(zpre, out_ap):
            """LN: bn_stats/bn_aggr (DVE) -> sqrt (ACT; unavoidable --
            Sqrt and Exp live in different ACT spline-table sets and the
            set loader is per-function-static, so the ~2 set switches per
            phase are the floor) -> reciprocal + normalize (DVE)."""
            stats = smallp.tile([128, 6], F32, tag="stats")
            mv = smallp.tile([128, 2], F32, tag="mv")
            nc.vector.bn_stats(out=stats, in_=zpre)
            nc.vector.bn_aggr(out=mv, in_=stats)
            stdv = smallp.tile([128, 1], F32, tag="stdv")
            nc.scalar.activation(out=stdv, in_=mv[:, 1:2],
                                 func=mybir.ActivationFunctionType.Sqrt,
                                 bias=eps_t, scale=1.0)
            rstd = smallp.tile([128, 1], F32, tag="rstd")
            nc.vector.reciprocal(out=rstd, in_=stdv)
            nc.vector.tensor_scalar(
                out=out_ap, in0=zpre, scalar1=mv[:, 0:1], scalar2=rstd,
                op0=SUB, op1=MULT)

        hT = [None] * NB    # [128, DC, NT] bf16 per batch (transposed h)
        h = [None] * NB     # RP pair tiles [128, 1024] bf16 per batch
        zs = [None] * NB
        zTs = [None] * NB

        def hsl(b, r):
            return h[b][r // 2][:, (r % 2) * DM:(r % 2 + 1) * DM]

        def phase1(l, b, lw, g1b, b1b, work=None):
            """QKV + scores/exp + Z + LN1 (+ per-pair z^T transposes) for
            one batch.  `work` is an ordered list of pending FFN-pair
            thunks (or None entries to skip a slot), consumed one per
            V-pair and one per Z-pair: drain bursts always overlap a
            stretch of independent matmuls, and cross-phase inputs (z^T /
            h^T transposes) are produced long before they are needed."""
            widx = [0]

            def slot():
                if work is not None and widx[0] < len(work):
                    thunk = work[widx[0]]
                    widx[0] += 1
                    if thunk is not None:
                        thunk()
            qT = [qkpool.tile([128, NT], BF16, tag="qk", name=f"qT{o}")
                  for o in range(DC)]
            kT = [qkpool.tile([128, NT], BF16, tag="qk", name=f"kT{o}")
                  for o in range(DC)]
            v2 = [vpool.tile([128, 2 * DM], BF16, tag="v2", name=f"v{p}")
                  for p in range(RP)]

            def vsl(c):
                return v2[c // 2][:, (c % 2) * DM:(c % 2 + 1) * DM]

            # QK chains: one [128,1024] PSUM tile per dm-chunk (both token
            # halves); full-tile drains alternate DVE/ACT.  (The h^T
            # transposes these chains depend on complete with lots of
            # slack now that the producing FFN is interleaved a phase
            # earlier, so no hf-ordering tricks are needed.)
            for dst, wname in ((qT, "wq"), (kT, "wk")):
                for o in range(DC):
                    ps = ps_big.tile([128, 1024], F32, tag="pb")
                    for hf in range(HF):
                        psl = ps[:, hf * 512:(hf + 1) * 512]
                        if l == 0:
                            w0n = "wq0" if wname == "wq" else "wk0"
                            nc.tensor.matmul(
                                psl,
                                w0[w0n][:, o * 128:(o + 1) * 128],
                                xts[b][:, hf * 512:(hf + 1) * 512],
                                start=True, stop=True)
                        else:
                            for i in range(DC):
                                nc.tensor.matmul(
                                    psl,
                                    lw[wname][:, i, o * 128:(o + 1) * 128],
                                    hT[b][:, i, hf * 512:(hf + 1) * 512],
                                    start=(i == 0), stop=(i == DC - 1))
                    nc.scalar.copy(out=dst[o], in_=ps)

            for p in range(RP):
                ps = ps_big.tile([128, 1024], F32, tag="pb")
                if l == 0:
                    for j in range(2):
                        r = 2 * p + j
                        nc.tensor.matmul(
                            ps[:, j * 512:(j + 1) * 512],
                            xts[b][:, r * 128:(r + 1) * 128], w0["wv0"],
                            start=True, stop=True)
                else:
                    for i in range(DC):
                        for j in range(2):
                            r = 2 * p + j
                            nc.tensor.matmul(
                                ps[:, j * 512:(j + 1) * 512],
                                hT[b][:, i, r * 128:(r + 1) * 128],
                                lw["wv"][:, i, :],
                                start=(i == 0), stop=(i == DC - 1))
                nc.scalar.copy(out=v2[p], in_=ps)
                slot()

            eT = []
            for c in range(RT):
                et = epool.tile([128, NT], BF16, tag="et")
                ats = ps_big.tile([128, 1024], F32, tag="pb")
                for i in range(DC):
                    for hf in range(HF):
                        nc.tensor.matmul(
                            ats[:, hf * 512:(hf + 1) * 512],
                            kT[i][:, c * 128:(c + 1) * 128],
                            qT[i][:, hf * 512:(hf + 1) * 512],
                            start=(i == 0), stop=(i == DC - 1))
                nc.scalar.activation(
                    out=et, in_=ats,
                    func=mybir.ActivationFunctionType.Exp, scale=ISQ)
                eT.append(et)

            z2 = [zpool.tile([128, 2 * DM], BF16, tag="z2", name=f"z{p}")
                  for p in range(RP)]
            zT = tchunk.tile([128, DC, NT], BF16, tag="tchunk", name="zT")
            # publish before the pair loop: after_pair callbacks for THIS
            # batch (last layer) read zTs[b]/zs[b] slices produced pairs
            # earlier; tile-slice dependencies handle readiness
            zs[b] = z2
            zTs[b] = zT
            for p in range(RP):
                r0 = 2 * p
                zps = ps_big.tile([128, 1024], F32, tag="pb")
                sp = [None, None]
                if l > 0:
                    sp = [ps_s.tile([128, 1], F32, tag="sp",
                                    name=f"sp{j}") for j in range(2)]
                for c in range(RT):
                    for j in range(2):
                        r = r0 + j
                        nc.tensor.matmul(
                            zps[:, j * 512:(j + 1) * 512],
                            eT[c][:, r * 128:(r + 1) * 128],
                            vsl(c), start=(c == 0), stop=(c == RT - 1))
                        if l > 0:
                            nc.tensor.matmul(
                                sp[j],
                                eT[c][:, r * 128:(r + 1) * 128], ones,
                                start=(c == 0), stop=(c == RT - 1))
                rs = None
                if l > 0:
                    rs = smallp.tile([128, 2], F32, tag="rs")
                    for j in range(2):
                        nc.vector.reciprocal(out=rs[:, j:j + 1], in_=sp[j])
                for j in range(2):
                    r = r0 + j
                    zsl = zps[:, j * 512:(j + 1) * 512]
                    osl = z2[p][:, j * 512:(j + 1) * 512]
                    zpre = tmpp.tile([128, DM], BF16, tag="zpre")
                    if affine_identity:
                        # fused drain: zpre = (Z * 1/s) + h (layer 0:
                        # + 0, 1/s folds into LN scale invariance)
                        nc.vector.scalar_tensor_tensor(
                            out=zpre, in0=zsl,
                            scalar=(rs[:, j:j + 1] if l > 0 else 1.0),
                            in1=(hsl(b, r) if l > 0 else warm),
                            op0=MULT, op1=ADD)
                        ln_a(zpre, osl)
                    else:
                        if l > 0:
                            tmp = tmpp.tile([128, DM], BF16, tag="tmp")
                            nc.scalar.activation(
                                out=tmp, in_=zsl,
                                func=mybir.ActivationFunctionType.Copy,
                                scale=rs[:, j:j + 1])
                            nc.vector.tensor_add(out=zpre, in0=tmp,
                                                 in1=hsl(b, r))
                        else:
                            nc.vector.tensor_copy(out=zpre, in_=zsl)
                        layernorm_r(zpre, osl, g1b, b1b)
                for j in range(2):
                    r = r0 + j
                    nc.sync.dma_start_transpose(
                        out=zT[:, :, r * 128:(r + 1) * 128],
                        in_=z2[p][:, (r % 2) * DM:(r % 2 + 1) * DM])
                slot()

        ffn_leftover = [None]   # callable emitting prev layer's FFN(b1) pair

        for l in range(3):
            lw = lws[l]
            if affine_identity:
                g1b = b1b = g2b = b2b = None
            else:
                g1b = load_ln("g1", l)
                b1b = load_ln("b1", l)
                g2b = load_ln("g2", l)
                b2b = load_ln("b2", l)
            bfb = None if bf_zero else load_ln("bf", l)

            def ffn_pair(b, p, g2t, nhT, lw=lw, bfb=bfb, g2b=g2b, b2b=b2b,
                         lpin=l, act_ln=False):
                r0 = 2 * p
                fps = ps_big.tile([128, 1024], F32, tag="pb")
                for i in range(DC):
                    for j in range(2):
                        r = r0 + j
                        nc.tensor.matmul(
                            fps[:, j * 512:(j + 1) * 512],
                            zTs[b][:, i, r * 128:(r + 1) * 128],
                            lw["wf"][:, i, :],
                            start=(i == 0), stop=(i == DC - 1))
                if bfb is not None:
                    for j in range(2):
                        nc.vector.tensor_add(
                            out=fps[:, j * 512:(j + 1) * 512],
                            in0=fps[:, j * 512:(j + 1) * 512], in1=bfb)
                for j in range(2):
                    fsl = fps[:, j * 512:(j + 1) * 512]
                    osl = g2t[:, j * 512:(j + 1) * 512]
                    gpre = tmpp.tile([128, DM], BF16, tag="gpre")
                    if affine_identity:
                        # fused drain: gpre = max(f, 0) + z
                        if act_ln:
                            # kernel-tail pair: sum via accum, variance
                            # via ACT Square so the trailing LN leans on
                            # the otherwise-idle ACT instead of the DVE
                            acc = smallp.tile([128, 2], F32, tag="facc")
                            nc.vector.scalar_tensor_tensor(
                                out=gpre, in0=fsl, scalar=0.0,
                                in1=zs[b][p][:, j * 512:(j + 1) * 512],
                                op0=MAX, op1=ADD, accum_out=acc[:, 0:1])
                            sqv = tmpp.tile([128, DM], BF16, tag="sqv")
                            nc.scalar.activation(
                                out=sqv, in_=gpre,
                                func=mybir.ActivationFunctionType.Square,
                                accum_out=acc[:, 1:2])
                            negmsq = smallp.tile([128, 1], F32,
                                                 tag="negmsq")
                            nc.vector.tensor_scalar(
                                out=negmsq, in0=acc[:, 0:1],
                                scalar1=acc[:, 0:1],
                                scalar2=-(IDM * IDM), op0=MULT, op1=MULT)
                            veps = smallp.tile([128, 1], F32, tag="veps")
                            nc.vector.tensor_scalar(
                                out=veps, in0=acc[:, 1:2], scalar1=IDM,
                                scalar2=negmsq, op0=MULT, op1=ADD)
                            stdv = smallp.tile([128, 1], F32, tag="stdv")
                            nc.scalar.activation(
                                out=stdv, in_=veps,
                                func=mybir.ActivationFunctionType.Sqrt,
                                bias=eps_t, scale=1.0)
                            rstd = smallp.tile([128, 1], F32, tag="rstd")
                            nc.vector.reciprocal(out=rstd, in_=stdv)
                            m = smallp.tile([128, 1], F32, tag="m")
                            nc.vector.tensor_scalar_mul(
                                out=m, in0=acc[:, 0:1], scalar1=IDM)
                            nc.vector.tensor_scalar(
                                out=osl, in0=gpre, scalar1=m,
                                scalar2=rstd, op0=SUB, op1=MULT)
                        else:
                            nc.vector.scalar_tensor_tensor(
                                out=gpre, in0=fsl, scalar=0.0,
                                in1=zs[b][p][:, j * 512:(j + 1) * 512],
                                op0=MAX, op1=ADD)
                            ln_a(gpre, osl)
                    else:
                        f_r = tmpp.tile([128, DM], BF16, tag="fr")
                        nc.scalar.activation(
                            out=f_r, in_=fsl,
                            func=mybir.ActivationFunctionType.Relu)
                        nc.vector.tensor_add(
                            out=gpre, in0=f_r,
                            in1=zs[b][p][:, j * 512:(j + 1) * 512])
                        layernorm_r(gpre, osl, g2b, b2b)
                if lpin == 2:
                    # one [256,512] store per pair on the idle gpsimd
                    # SWDGE queue (keeps the tail off the HWDGE queues)
                    nc.gpsimd.dma_start(
                        out=d["out"][b * NT + r0 * 128:
                                     b * NT + (r0 + 2) * 128, :]
                        .rearrange("(j p) o -> p j o", j=2),
                        in_=g2t.rearrange("p (j o) -> p j o", j=2))
                else:
                    for j in range(2):
                        r = r0 + j
                        nc.sync.dma_start_transpose(
                            out=nhT[:, :, r * 128:(r + 1) * 128],
                            in_=g2t[:, j * 512:(j + 1) * 512])

            # phase 1 for batch 0 consumes the previous layer's leftover
            # FFN(b1) pairs (all their inputs are long since ready);
            # phase 1 for batch 1 consumes this layer's FFN(b0) pairs at
            # its V slots, so the h^T(b0) transposes complete well before
            # the next layer's QK chains.  FFN(b1) becomes the next
            # layer's leftover; for the last layer its first three pairs
            # ride in batch 1's Z slots and only pair 3 trails the kernel.
            phase1(l, 0, lw, g1b, b1b, work=ffn_leftover[0])
            ffn_leftover[0] = None

            def mk(b, p, g2t, nhT, ffn_pair=ffn_pair):
                return lambda: ffn_pair(b, p, g2t, nhT)

            if l < 2:
                nhT0 = tchunk.tile([128, DC, NT], BF16, tag="tchunk",
                                   name="hT0")
                ng0 = [gpool.tile([128, 2 * DM], BF16, tag="g2",
                                  name=f"g0_{p}") for p in range(RP)]
                phase1(l, 1, lw, g1b, b1b,
                       work=[mk(0, p, ng0[p], nhT0) for p in range(RP)])
                nhT1 = tchunk.tile([128, DC, NT], BF16, tag="tchunk",
                                   name="hT1")
                ng1 = [gpool.tile([128, 2 * DM], BF16, tag="g2",
                                  name=f"g1_{p}") for p in range(RP)]
                ffn_leftover[0] = [mk(1, p, ng1[p], nhT1) for p in range(RP)]
                hT[0], hT[1] = nhT0, nhT1
                h[0], h[1] = ng0, ng1
            else:
                gts = [gout.tile([128, 2 * DM], BF16, tag="gout",
                                 name=f"go_{p}") for p in range(RP)]
                gts1 = [gout.tile([128, 2 * DM], BF16, tag="gout",
                                  name=f"go1_{p}") for p in range(RP)]
                # V slots: FFN(b0) p0-p3; Z slot 0 skipped (pair 0's z^T
                # transposes enter the queue only at that slot); Z slots
                # 1-3: FFN(b1) p0-p2; FFN(b1) p3 is the kernel tail.
                phase1(l, 1, lw, g1b, b1b,
                       work=[mk(0, p, gts[p], None) for p in range(RP)]
                       + [None]
                       + [mk(1, p, gts1[p], None) for p in range(RP - 1)])
                ffn_pair(1, RP - 1, gts1[RP - 1], None, act_ln=True)


def kernel(**inputs):
    x = np.asarray(inputs["x"], np.float32)          # [16, 1024, 64]
    bfdt = np.dtype(mybir.dt.np(BF16))

    def to_bf16(a):
        return np.ascontiguousarray(np.asarray(a, np.float32).astype(bfdt))

    shared = {
        "wq0": to_bf16(inputs["Wq0"]),
        "wk0": to_bf16(inputs["Wk0"]),
        "wv0": to_bf16(inputs["Wv0"]),
        "wqs": to_bf16(inputs["Wqs"]),
        "wks": to_bf16(inputs["Wks"]),
        "wvs": to_bf16(inputs["Wvs"]),
        "wf": to_bf16(inputs["Wf"]),
        "g1": np.ascontiguousarray(inputs["g1"], np.float32),
        "b1": np.ascontiguousarray(inputs["b1"], np.float32),
        "g2": np.ascontiguousarray(inputs["g2"], np.float32),
        "b2": np.ascontiguousarray(inputs["b2"], np.float32),
        "bf": np.ascontiguousarray(inputs["bf"], np.float32),
    }
    in_maps = []
    for i in range(NCORES):
        xt = to_bf16(
            np.concatenate([x[NB * i + b].T for b in range(NB)], axis=1))
        m = dict(shared)
        m["xt"] = xt
        in_maps.append(m)

    affine_identity = bool(
        np.all(shared["g1"] == 1) and np.all(shared["b1"] == 0)
        and np.all(shared["g2"] == 1) and np.all(shared["b2"] == 0))
    bf_zero = bool(np.all(shared["bf"] == 0))

    nc = bacc.Bacc()
    build_graph(nc, affine_identity=affine_identity, bf_zero=bf_zero)
    res = run_bass_kernel_spmd(nc, in_maps, list(range(NCORES)), trace=TRACE)
    if TRACE:
        print("exec_time_ns:", res.exec_time_ns, "mean:", res.mean_exec_time_ns)
        kernel.last_result = res

    y = np.empty((NCORES * NB, NT, DM), np.float32)
    for i in range(NCORES):
        o = np.asarray(res.results[i]["out"]).astype(np.float32)
        for b in range(NB):
            y[NB * i + b] = o[b * NT:(b + 1) * NT]
    return y
